# revision 6
# baseline (speedup 1.0000x reference)
"""Trainium2 Bass kernel for ChannelAttention.

Reference computation (B=32, N=784, C=768, G=8 groups of CH=96 channels):
    qkv  = x @ w_qkv.T + b_qkv                    # [B,N,3C]
    q,k,v split into G groups of CH channels; q *= N**-0.5
    attn = softmax(einsum('bgnc,bgnd->bgcd', q, k), axis=-1)
    out  = einsum('bgcd,bgnd->bgnc', attn, v)     # [B,G,N,CH] -> [B,N,C]
    out  = out @ w_proj.T + b_proj
    returns (out, size)   # size passes through

Sharding: data-parallel over batch, B/8 = 4 batches per NeuronCore, no
collectives. Weights replicated (host pre-transposes them so both matmul
operands have the contraction dim on SBUF partitions).

Per-core dataflow (per batch):
  X:  DMA x[b] naturally, PE-transpose 128x128 blocks -> xT [C, N]
  QK: qkv matmul over 6 c-tiles -> q,k stored [n, j] (tokens on partitions)
  V:  same inputs, roles swapped -> vT stored group-aligned [d, n]
  A:  attnT[d,c] = k^T q (contract n); exp via ACT (scale=N^-0.5, no max
      subtraction needed: logits ~ N(0,1)); colsum via matmul with ones
      -> [c,1]; reciprocal; out_cT[c,n] = expT^T @ vT with the softmax
      normalization folded into the PSUM evacuation (per-partition scalar)
  P:  proj matmul contracting per-group (K=96, 8 groups) -> y [n, c] -> DMA

Matmul dtypes: float32r (full-rate fp32 mode) for QK/V/P, bf16 for the
attention block (small free dims where fp32r runs at 1/4 rate).
"""

import math

import numpy as np

import concourse.bass as bass
import concourse.mybir as mybir
from concourse import bacc
from concourse.bass_utils import run_bass_kernel_spmd
from concourse.masks import make_identity
from concourse.tile import TileContext

B, N, C, G = 32, 784, 768, 8
CH = C // G  # 96
NCORES = 8
BP = B // NCORES  # batches per core
P = 128
CT = C // P  # 6 contraction tiles for qkv/x
NT = (N + P - 1) // P  # 7 token tiles (last has 16 rows)
JCH = 384  # free-dim chunk for the q/k projection (2C = 4 chunks)
NCH = 392  # token chunk for v / attn-out phases (N = 2 chunks)
COCH = 384  # output-channel chunk for proj (C = 2 chunks)
SCALE = 1.0 / math.sqrt(N)

F32 = mybir.dt.float32
F32R = mybir.dt.float32r
BF16 = mybir.dt.bfloat16

EXP = mybir.ActivationFunctionType.Exp


def _rows(t):
    return min(P, N - t * P)


def build_nc(mm="f32r", attn="bf16"):
    """Build the per-core Bass module.

    mm:   dtype mode for the qkv / proj matmuls ('f32r' | 'f32' | 'bf16')
    attn: dtype mode for q/k/v storage + attention matmuls ('bf16' | 'f32' | 'f32r')
    """
    # fp32r tiles must be *produced* as fp32r (DMA from an fp32r DRAM
    # tensor, or a compute op writing an fp32r-typed tile) — the walrus
    # verifier rejects plain-fp32 producers feeding fp32r matmuls.
    if mm == "f32r":
        wdt = F32R
    elif mm == "bf16":
        wdt = BF16
    else:
        wdt = F32
    adt = F32R if attn == "f32r" else (BF16 if attn == "bf16" else F32)

    def mmcast(ap):
        return ap

    def acast(ap):
        return ap

    nc = bacc.Bacc("TRN2", target_bir_lowering=False, debug=False)
    x = nc.dram_tensor("x", [BP, N, C], F32, kind="ExternalInput")
    wqkvT = nc.dram_tensor("wqkvT", [C, 3 * C], wdt, kind="ExternalInput")
    bqkv = nc.dram_tensor("bqkv", [3 * C], F32, kind="ExternalInput")
    wprojT = nc.dram_tensor("wprojT", [C, C], wdt, kind="ExternalInput")
    bproj = nc.dram_tensor("bproj", [C], F32, kind="ExternalInput")
    y = nc.dram_tensor("y", [BP, N, C], F32, kind="ExternalOutput")

    from contextlib import ExitStack

    with TileContext(nc) as tc, ExitStack() as ctx:
        const = ctx.enter_context(tc.tile_pool(name="const", bufs=1))
        wpool = ctx.enter_context(tc.tile_pool(name="w", bufs=1))
        xnat_p = ctx.enter_context(tc.tile_pool(name="xnat", bufs=3))
        xT_p = ctx.enter_context(tc.tile_pool(name="xT", bufs=1))
        qk_p = ctx.enter_context(tc.tile_pool(name="qk", bufs=1))
        v_p = ctx.enter_context(tc.tile_pool(name="v", bufs=1))
        a_p = ctx.enter_context(tc.tile_pool(name="a", bufs=4))
        oT_p = ctx.enter_context(tc.tile_pool(name="oT", bufs=1))
        y_p = ctx.enter_context(tc.tile_pool(name="y", bufs=3))
        ps_big = ctx.enter_context(tc.tile_pool(name="ps_big", bufs=6, space="PSUM"))
        ps_a = ctx.enter_context(tc.tile_pool(name="ps_a", bufs=1, space="PSUM"))

        if True:
            # ---- constants ----
            ident = const.tile([P, P], F32)
            make_identity(nc, ident)
            ones_col = const.tile([P, 1], adt)
            nc.vector.memset(ones_col, 1.0)

            # bias broadcast across partitions for q/k ([0,2C)) and proj
            bias_qk = const.tile([P, 2 * C], F32)
            src = bqkv[0 : 2 * C]
            nc.gpsimd.dma_start(
                out=bias_qk,
                in_=bass.AP(src.tensor, src.offset, [[0, P], [1, 2 * C]]),
            )
            bias_pj = const.tile([P, C], F32)
            src = bproj[0:C]
            nc.gpsimd.dma_start(
                out=bias_pj,
                in_=bass.AP(src.tensor, src.offset, [[0, P], [1, C]]),
            )
            # per-partition bias for v: element [d, g] = bqkv[2C + g*CH + d]
            bias_v = const.tile([CH, G], F32)
            src = bqkv[:]
            nc.gpsimd.dma_start(
                out=bias_v,
                in_=bass.AP(src.tensor, 2 * C, [[1, CH], [CH, G]]),
            )

            # ---- weights (host passed transposed: [c_in, j]) ----
            wqkvT_sb = wpool.tile([P, CT, 3 * C], wdt)
            for ck in range(CT):
                nc.sync.dma_start(
                    out=wqkvT_sb[:, ck, :], in_=wqkvT[ck * P : (ck + 1) * P, :]
                )
            # proj weights stored group-aligned: rows [g*CH, (g+1)*CH)
            wprojT_sb = wpool.tile([CH, G, C], wdt)
            for g in range(G):
                nc.sync.dma_start(
                    out=wprojT_sb[:, g, :], in_=wprojT[g * CH : (g + 1) * CH, :]
                )

            for b in range(BP):
                # ---- X: load + transpose x[b] -> xT [c, n] ----
                xT_sb = xT_p.tile([P, CT, N], wdt, tag="xT")
                for t in range(NT):
                    r = _rows(t)
                    xt = xnat_p.tile([P, C], F32, tag="xnat")
                    nc.sync.dma_start(
                        out=xt[:r, :], in_=x[b, t * P : t * P + r, :]
                    )
                    for ck in range(CT):
                        pt = ps_big.tile([P, P], F32, tag="big")
                        nc.tensor.transpose(
                            pt[:, :r],
                            xt[:r, ck * P : (ck + 1) * P],
                            ident[:r, :r],
                        )
                        nc.vector.tensor_copy(
                            out=xT_sb[:, ck, t * P : t * P + r], in_=pt[:, :r]
                        )

                # ---- QK: q,k = x @ w_qkv[:2C].T, stored [n, j] ----
                q_sb = qk_p.tile([P, NT, C], adt, tag="q")
                k_sb = qk_p.tile([P, NT, C], adt, tag="k")
                for t in range(NT):
                    r = _rows(t)
                    for jc in range(2 * C // JCH):
                        ps = ps_big.tile([P, JCH], F32, tag="big")
                        for ck in range(CT):
                            nc.tensor.matmul(
                                ps[:r, :],
                                mmcast(xT_sb[:, ck, t * P : t * P + r]),
                                mmcast(
                                    wqkvT_sb[:, ck, jc * JCH : (jc + 1) * JCH]
                                ),
                                start=(ck == 0),
                                stop=(ck == CT - 1),
                            )
                        if jc < C // JCH:
                            dst, off = q_sb, jc * JCH
                        else:
                            dst, off = k_sb, jc * JCH - C
                        nc.vector.tensor_add(
                            out=dst[:r, t, off : off + JCH],
                            in0=ps[:r, :],
                            in1=bias_qk[:r, jc * JCH : (jc + 1) * JCH],
                        )

                # ---- V: vT = (x @ w_qkv[2C:].T).T, group-aligned [d, n] ----
                vT_sb = v_p.tile([CH, G, N], adt, tag="vT")
                for g in range(G):
                    for nch in range(N // NCH):
                        n0 = nch * NCH
                        ps = ps_big.tile([CH, NCH], F32, tag="big")
                        for ck in range(CT):
                            nc.tensor.matmul(
                                ps,
                                mmcast(
                                    wqkvT_sb[
                                        :,
                                        ck,
                                        2 * C + g * CH : 2 * C + (g + 1) * CH,
                                    ]
                                ),
                                mmcast(xT_sb[:, ck, n0 : n0 + NCH]),
                                start=(ck == 0),
                                stop=(ck == CT - 1),
                            )
                        nc.vector.tensor_scalar_add(
                            out=vT_sb[:, g, n0 : n0 + NCH],
                            in0=ps,
                            scalar1=bias_v[:, g : g + 1],
                        )

                # ---- A: channel attention per group ----
                outT_sb = oT_p.tile([CH, G, N], wdt, tag="outT")
                for g in range(G):
                    psA = ps_a.tile([CH, CH], F32, tag="attnT")
                    for t in range(NT):
                        r = _rows(t)
                        nc.tensor.matmul(
                            psA,
                            acast(k_sb[:r, t, g * CH : (g + 1) * CH]),
                            acast(q_sb[:r, t, g * CH : (g + 1) * CH]),
                            start=(t == 0),
                            stop=(t == NT - 1),
                        )
                    expT = a_p.tile([CH, CH], adt, tag="expT")
                    nc.scalar.activation(
                        out=expT, in_=psA, func=EXP, scale=SCALE
                    )
                    psS = ps_a.tile([CH, 1], F32, tag="colsum")
                    nc.tensor.matmul(
                        psS,
                        acast(expT),
                        acast(ones_col[:CH, :]),
                        start=True,
                        stop=True,
                    )
                    recip = a_p.tile([CH, 1], F32, tag="recip")
                    nc.vector.reciprocal(out=recip, in_=psS)
                    for nch in range(N // NCH):
                        n0 = nch * NCH
                        psO = ps_big.tile([CH, NCH], F32, tag="big")
                        nc.tensor.matmul(
                            psO,
                            acast(expT),
                            acast(vT_sb[:, g, n0 : n0 + NCH]),
                            start=True,
                            stop=True,
                        )
                        nc.vector.tensor_scalar_mul(
                            out=outT_sb[:, g, n0 : n0 + NCH],
                            in0=psO,
                            scalar1=recip,
                        )

                # ---- P: y = out @ w_proj.T + b_proj, stored [n, c] ----
                for t in range(NT):
                    r = _rows(t)
                    y_sb = y_p.tile([P, C], F32, tag="y")
                    for cc in range(C // COCH):
                        ps = ps_big.tile([P, COCH], F32, tag="big")
                        for g in range(G):
                            nc.tensor.matmul(
                                ps[:r, :],
                                mmcast(outT_sb[:, g, t * P : t * P + r]),
                                mmcast(
                                    wprojT_sb[:, g, cc * COCH : (cc + 1) * COCH]
                                ),
                                start=(g == 0),
                                stop=(g == G - 1),
                            )
                        nc.vector.tensor_add(
                            out=y_sb[:r, cc * COCH : (cc + 1) * COCH],
                            in0=ps[:r, :],
                            in1=bias_pj[:r, cc * COCH : (cc + 1) * COCH],
                        )
                    nc.sync.dma_start(
                        out=y[b, t * P : t * P + r, :], in_=y_sb[:r, :]
                    )

    nc.compile()
    return nc


_CACHE = {}
LAST_RESULTS = None


def _get_nc(mm, attn):
    key = (mm, attn)
    if key not in _CACHE:
        _CACHE[key] = build_nc(mm=mm, attn=attn)
    return _CACHE[key]


def _make_in_maps(inputs, mm):
    x = np.ascontiguousarray(np.asarray(inputs["x"], dtype=np.float32))
    w_qkv = np.asarray(inputs["w_qkv"], dtype=np.float32)
    b_qkv = np.ascontiguousarray(np.asarray(inputs["b_qkv"], dtype=np.float32))
    w_proj = np.asarray(inputs["w_proj"], dtype=np.float32)
    b_proj = np.ascontiguousarray(np.asarray(inputs["b_proj"], dtype=np.float32))

    wdt = np.float32
    if mm == "bf16":
        import ml_dtypes

        wdt = ml_dtypes.bfloat16
    wqkvT = np.ascontiguousarray(w_qkv.T).astype(wdt)
    wprojT = np.ascontiguousarray(w_proj.T).astype(wdt)

    return [
        {
            "x": np.ascontiguousarray(x[c * BP : (c + 1) * BP]),
            "wqkvT": wqkvT,
            "bqkv": b_qkv,
            "wprojT": wprojT,
            "bproj": b_proj,
        }
        for c in range(NCORES)
    ]


def kernel(**inputs):
    global LAST_RESULTS
    import os

    mm = os.environ.get("KERNEL_MM_DTYPE", "f32r")
    attn = os.environ.get("KERNEL_ATTN_DTYPE", "bf16")

    nc = _get_nc(mm, attn)
    in_maps = _make_in_maps(inputs, mm)
    res = run_bass_kernel_spmd(nc, in_maps, core_ids=list(range(NCORES)))
    LAST_RESULTS = res
    out = np.concatenate([r["y"] for r in res.results], axis=0)
    return out, np.asarray(inputs["size"])


# revision 30
# speedup vs baseline: 1.0278x; 1.0278x over previous
"""Trainium2 Bass kernel for ChannelAttention.

Reference computation (B=32, N=784, C=768, G=8 groups of CH=96 channels):
    qkv  = x @ w_qkv.T + b_qkv                    # [B,N,3C]
    q,k,v split into G groups of CH channels; q *= N**-0.5
    attn = softmax(einsum('bgnc,bgnd->bgcd', q, k), axis=-1)
    out  = einsum('bgcd,bgnd->bgnc', attn, v)     # [B,G,N,CH] -> [B,N,C]
    out  = out @ w_proj.T + b_proj
    returns (out, size)   # size passes through

Sharding: data-parallel over batch, B/8 = 4 batches per NeuronCore, no
collectives. Weights replicated (host pre-transposes them so both matmul
operands have the contraction dim on SBUF partitions).

Per-core dataflow (per batch):
  X:  DMA x[b] naturally, PE-transpose 128x128 blocks -> xT [C, N]
  QK: qkv matmul over 6 c-tiles -> q,k stored [n, j] (tokens on partitions)
  V:  same inputs, roles swapped -> vT stored group-aligned [d, n]
  A:  attnT[d,c] = k^T q (contract n); exp via ACT (scale=N^-0.5, no max
      subtraction needed: logits ~ N(0,1)); colsum via matmul with ones
      -> [c,1]; reciprocal; out_cT[c,n] = expT^T @ vT with the softmax
      normalization folded into the PSUM evacuation (per-partition scalar)
  P:  proj matmul contracting per-group (K=96, 8 groups) -> y [n, c] -> DMA

Matmul dtypes: float32r (full-rate fp32 mode) for QK/V/P, bf16 for the
attention block (small free dims where fp32r runs at 1/4 rate).
"""

import math

import numpy as np

import concourse.bass as bass
import concourse.mybir as mybir
from concourse import bacc
from concourse.bass_utils import run_bass_kernel_spmd
from concourse.masks import make_identity
from concourse.tile import TileContext

B, N, C, G = 32, 784, 768, 8
CH = C // G  # 96
NCORES = 8
BP = B // NCORES  # batches per core
P = 128
CT = C // P  # 6 contraction tiles for qkv/x
NT = (N + P - 1) // P  # 7 token tiles (last has 16 rows)
JCH = 384  # free-dim chunk for the q/k projection (2C = 4 chunks)
NCH = 392  # token chunk for v / attn-out phases (N = 2 chunks)
COCH = 384  # output-channel chunk for proj (C = 2 chunks)
SCALE = 1.0 / math.sqrt(N)

F32 = mybir.dt.float32
F32R = mybir.dt.float32r
BF16 = mybir.dt.bfloat16

EXP = mybir.ActivationFunctionType.Exp


def _rows(t):
    return min(P, N - t * P)


def build_nc(
    mm="bf16",
    attn="bf16",
    proj128=True,
    v128=True,
    xconv_pool=True,
    biases=True,
    jch=512,
    reps=None,
):
    """Build the per-core Bass module.

    mm:      dtype mode for the qkv / proj matmuls ('f32r' | 'f32' | 'bf16')
    attn:    dtype mode for q/k/v storage + attention matmuls ('bf16'|'f32'|'f32r')
    proj128: repack the attention output into a 128-aligned layout (SBUF->SBUF
             DMA) so proj contracts with K=128 x 6 instead of K=96 x 8
    v128:    compute v with 128-wide stationary tiles (full PE density), then
             repack group-aligned via SBUF->SBUF DMA
    xconv_pool: run the x fp32->bf16 pre-transpose conversion on GpSimd
    biases:  emit the (qkv/proj) bias adds; when all biases are zero the host
             builds the kernel with biases=False and evacuations become copies
    jch:     free-dim chunk for the q/k projection matmuls
    reps:    if set, wrap the whole body in a hardware For_i loop (timing
             amplification: device time becomes visible above the ~90 ms
             axon dispatch floor)
    """
    # fp32r tiles must be *produced* as fp32r (DMA from an fp32r DRAM
    # tensor, or a compute op writing an fp32r-typed tile) -- the walrus
    # verifier rejects plain-fp32 producers feeding fp32r matmuls.
    if mm == "f32r":
        wdt = F32R
    elif mm == "bf16":
        wdt = BF16
    else:
        wdt = F32
    adt = F32R if attn == "f32r" else (BF16 if attn == "bf16" else F32)
    # dtype of the transpose input (and identity): converting x to bf16
    # before the PE transpose runs it at 1 cycle/row instead of 2.
    tdt = BF16 if mm == "bf16" else F32

    nc = bacc.Bacc("TRN2", target_bir_lowering=False, debug=False)
    x = nc.dram_tensor("x", [BP, N, C], F32, kind="ExternalInput")
    wqkvT = nc.dram_tensor("wqkvT", [C, 3 * C], wdt, kind="ExternalInput")
    bqkv = nc.dram_tensor("bqkv", [3 * C], F32, kind="ExternalInput")
    wprojT = nc.dram_tensor("wprojT", [C, C], wdt, kind="ExternalInput")
    bproj = nc.dram_tensor("bproj", [C], F32, kind="ExternalInput")
    y = nc.dram_tensor("y", [BP, N, C], F32, kind="ExternalOutput")

    from contextlib import ExitStack

    with TileContext(nc) as tc, ExitStack() as ctx:
        const = ctx.enter_context(tc.tile_pool(name="const", bufs=1))
        wpool = ctx.enter_context(tc.tile_pool(name="w", bufs=1))
        xnat_p = ctx.enter_context(tc.tile_pool(name="xnat", bufs=3))
        xT_p = ctx.enter_context(
            tc.tile_pool(name="xT", bufs=2 if mm == "bf16" else 1)
        )
        qk_p = ctx.enter_context(tc.tile_pool(name="qk", bufs=1))
        v_p = ctx.enter_context(tc.tile_pool(name="v", bufs=1))
        a_p = ctx.enter_context(tc.tile_pool(name="a", bufs=G + 1))
        oT_p = ctx.enter_context(tc.tile_pool(name="oT", bufs=1))
        y_p = ctx.enter_context(tc.tile_pool(name="y", bufs=3))
        ps_big = ctx.enter_context(tc.tile_pool(name="ps_big", bufs=6, space="PSUM"))
        ps_a = ctx.enter_context(tc.tile_pool(name="ps_a", bufs=1, space="PSUM"))

        from contextlib import nullcontext

        with tc.For_i(0, reps, 1) if reps else nullcontext():
            # ---- constants ----
            ident = const.tile([P, P], tdt)
            make_identity(nc, ident)
            ones_col = const.tile([P, 1], adt)
            nc.vector.memset(ones_col, 1.0)

            if biases:
                # bias broadcast across partitions for q/k ([0,2C)) and proj
                bias_qk = const.tile([P, 2 * C], F32)
                src = bqkv[0 : 2 * C]
                nc.gpsimd.dma_start(
                    out=bias_qk,
                    in_=bass.AP(src.tensor, src.offset, [[0, P], [1, 2 * C]]),
                )
                bias_pj = const.tile([P, C], F32)
                src = bproj[0:C]
                nc.gpsimd.dma_start(
                    out=bias_pj,
                    in_=bass.AP(src.tensor, src.offset, [[0, P], [1, C]]),
                )
                # per-partition bias for v: [d, jt] = bqkv[2C + jt*W + d]
                vw = P if v128 else CH
                bias_v = const.tile([vw, C // vw], F32)
                src = bqkv[:]
                nc.gpsimd.dma_start(
                    out=bias_v,
                    in_=bass.AP(src.tensor, 2 * C, [[1, vw], [vw, C // vw]]),
                )

            # ---- X phase helper (batch 0 is emitted before the weight DMAs
            # so the PE has transpose work while weights stream in) ----
            def emit_X(b):
                xT_sb = xT_p.tile([P, CT, N], wdt, tag="xT")
                for t in range(NT):
                    r = _rows(t)
                    xt = xnat_p.tile([P, C], F32, tag="xnat")
                    nc.sync.dma_start(
                        out=xt[:r, :], in_=x[b, t * P : t * P + r, :]
                    )
                    if mm == "bf16":
                        xin = xnat_p.tile([P, C], BF16, tag="xbf")
                        conv = (
                            nc.gpsimd.tensor_copy
                            if xconv_pool
                            else nc.vector.tensor_copy
                        )
                        conv(out=xin[:r, :], in_=xt[:r, :])
                    else:
                        xin = xt
                    # transpose pairs of 128-blocks into one PSUM tile and
                    # evacuate both with a single ACT copy (DVE is the busy
                    # engine; ACT only runs Exp otherwise)
                    for ckp in range(CT // 2):
                        pt = ps_big.tile([P, 2, P], tdt, tag="big")
                        for h in range(2):
                            ck = 2 * ckp + h
                            nc.tensor.transpose(
                                pt[:, h, :r],
                                xin[:r, ck * P : (ck + 1) * P],
                                ident[:r, :r],
                            )
                        nc.scalar.copy(
                            out=xT_sb[:, 2 * ckp : 2 * ckp + 2, t * P : t * P + r],
                            in_=pt[:, :, :r],
                        )
                return xT_sb

            xT_next = emit_X(0)

            # ---- weights (host passed transposed: [c_in, j]); one tile per
            # contraction slice so matmuls only wait for their own slice ----
            wqkv_sb = []
            for ck in range(CT):
                wt = wpool.tile([P, 3 * C], wdt, tag=f"wqkv{ck}")
                nc.scalar.dma_start(
                    out=wt, in_=wqkvT[ck * P : (ck + 1) * P, :]
                )
                wqkv_sb.append(wt)
            if proj128:
                wproj_sb = []
                for ck in range(CT):
                    wt = wpool.tile([P, C], wdt, tag=f"wproj{ck}")
                    nc.scalar.dma_start(
                        out=wt, in_=wprojT[ck * P : (ck + 1) * P, :]
                    )
                    wproj_sb.append(wt)
            else:
                wproj_sb = []
                for g in range(G):
                    wt = wpool.tile([CH, C], wdt, tag=f"wproj{g}")
                    nc.scalar.dma_start(
                        out=wt, in_=wprojT[g * CH : (g + 1) * CH, :]
                    )
                    wproj_sb.append(wt)

            for b in range(BP):
                xT_sb = xT_next

                # ---- QK: q,k = x @ w_qkv[:2C].T, stored [n, j] ----
                q_sb = qk_p.tile([P, NT, C], adt, tag="q")
                k_sb = qk_p.tile([P, NT, C], adt, tag="k")
                for t in range(NT):
                    r = _rows(t)
                    for jc in range(2 * C // jch):
                        ps = ps_big.tile([P, jch], F32, tag="big")
                        for ck in range(CT):
                            nc.tensor.matmul(
                                ps[:r, :],
                                xT_sb[:, ck, t * P : t * P + r],
                                wqkv_sb[ck][:, jc * jch : (jc + 1) * jch],
                                start=(ck == 0),
                                stop=(ck == CT - 1),
                            )
                        # evacuate, splitting at the q|k boundary (j = C)
                        j0 = jc * jch
                        while j0 < (jc + 1) * jch:
                            j1 = min((jc + 1) * jch, C if j0 < C else 2 * C)
                            dst, off = (
                                (q_sb, j0) if j0 < C else (k_sb, j0 - C)
                            )
                            p0 = j0 - jc * jch
                            if biases:
                                nc.vector.tensor_add(
                                    out=dst[:r, t, off : off + (j1 - j0)],
                                    in0=ps[:r, p0 : p0 + (j1 - j0)],
                                    in1=bias_qk[:r, j0:j1],
                                )
                            else:
                                nc.vector.tensor_copy(
                                    out=dst[:r, t, off : off + (j1 - j0)],
                                    in_=ps[:r, p0 : p0 + (j1 - j0)],
                                )
                            j0 = j1

                # ---- V: vT = (x @ w_qkv[2C:].T).T ----
                # group-aligned copy consumed by the attention phase
                vT_sb = v_p.tile([CH, G, N], adt, tag="vT")
                if v128:
                    # full-density stationaries, then SBUF->SBUF DMA repack
                    vT128_sb = v_p.tile([P, CT, N], adt, tag="vT128")
                    for jt in range(CT):
                        for nch in range(N // NCH):
                            n0 = nch * NCH
                            ps = ps_big.tile([P, NCH], F32, tag="big")
                            for ck in range(CT):
                                nc.tensor.matmul(
                                    ps,
                                    wqkv_sb[ck][
                                        :, 2 * C + jt * P : 2 * C + (jt + 1) * P
                                    ],
                                    xT_sb[:, ck, n0 : n0 + NCH],
                                    start=(ck == 0),
                                    stop=(ck == CT - 1),
                                )
                            if biases:
                                nc.vector.tensor_scalar_add(
                                    out=vT128_sb[:, jt, n0 : n0 + NCH],
                                    in0=ps,
                                    scalar1=bias_v[:, jt : jt + 1],
                                )
                            else:
                                nc.vector.tensor_copy(
                                    out=vT128_sb[:, jt, n0 : n0 + NCH],
                                    in_=ps,
                                )
                    for g in range(G):
                        c0 = g * CH
                        seg = c0
                        while seg < c0 + CH:
                            ct_i, p0 = seg // P, seg % P
                            ln = min((ct_i + 1) * P, c0 + CH) - seg
                            o0 = seg - c0
                            nc.scalar.dma_start(
                                out=vT_sb[o0 : o0 + ln, g, :],
                                in_=vT128_sb[p0 : p0 + ln, ct_i, :],
                            )
                            seg += ln
                else:
                    for g in range(G):
                        for nch in range(N // NCH):
                            n0 = nch * NCH
                            ps = ps_big.tile([CH, NCH], F32, tag="big")
                            for ck in range(CT):
                                nc.tensor.matmul(
                                    ps,
                                    wqkv_sb[ck][
                                        :,
                                        2 * C + g * CH : 2 * C + (g + 1) * CH,
                                    ],
                                    xT_sb[:, ck, n0 : n0 + NCH],
                                    start=(ck == 0),
                                    stop=(ck == CT - 1),
                                )
                            if biases:
                                nc.vector.tensor_scalar_add(
                                    out=vT_sb[:, g, n0 : n0 + NCH],
                                    in0=ps,
                                    scalar1=bias_v[:, g : g + 1],
                                )
                            else:
                                nc.vector.tensor_copy(
                                    out=vT_sb[:, g, n0 : n0 + NCH], in_=ps
                                )

                # prefetch next batch's X phase (PE transposes fill gaps
                # while this batch's attention chain runs)
                if b + 1 < BP:
                    xT_next = emit_X(b + 1)

                # ---- A: channel attention, software-pipelined over groups ----
                # pass 1: logits + exp  (PE streams attnT(g+1) while ACT
                # exponentiates group g -- no PE stall on the ACT roundtrip)
                outT_sb = oT_p.tile([CH, G, N], wdt, tag="outT")
                if proj128:
                    outT128_sb = [
                        oT_p.tile(
                            [P, N], wdt, tag=f"outT128_{ck}", name=f"oT128_{ck}"
                        )
                        for ck in range(CT)
                    ]
                expTs, recips = [], []
                for g in range(G):
                    psA = ps_a.tile([CH, CH], F32, tag="attnT", bufs=1)
                    for t in range(NT):
                        r = _rows(t)
                        nc.tensor.matmul(
                            psA,
                            k_sb[:r, t, g * CH : (g + 1) * CH],
                            q_sb[:r, t, g * CH : (g + 1) * CH],
                            start=(t == 0),
                            stop=(t == NT - 1),
                        )
                    expT = a_p.tile([CH, CH], adt, tag="expT")
                    nc.scalar.activation(
                        out=expT, in_=psA, func=EXP, scale=SCALE
                    )
                    expTs.append(expT)
                # pass 2: softmax denominators
                for g in range(G):
                    psS = ps_a.tile([CH, 1], F32, tag="colsum")
                    nc.tensor.matmul(
                        psS, expTs[g], ones_col[:CH, :], start=True, stop=True
                    )
                    recip = a_p.tile([CH, 1], F32, tag="recip")
                    nc.vector.reciprocal(out=recip, in_=psS)
                    recips.append(recip)
                # pass 3: attention output; softmax normalization folded into
                # the PSUM evacuation (per-partition scalar multiply)
                for g in range(G):
                    for nch in range(N // NCH):
                        n0 = nch * NCH
                        psO = ps_big.tile([CH, NCH], F32, tag="big")
                        nc.tensor.matmul(
                            psO,
                            expTs[g],
                            vT_sb[:, g, n0 : n0 + NCH],
                            start=True,
                            stop=True,
                        )
                        nc.vector.tensor_scalar_mul(
                            out=outT_sb[:, g, n0 : n0 + NCH],
                            in0=psO,
                            scalar1=recips[g],
                        )
                    if proj128:
                        # repack rows [g*CH, (g+1)*CH) into the 128-aligned
                        # layout via SBUF->SBUF DMA (partition-address based,
                        # so it can do the 96->128 shift that DVE cannot)
                        c0 = g * CH
                        seg = c0
                        while seg < c0 + CH:
                            ct_i, p0 = seg // P, seg % P
                            ln = min((ct_i + 1) * P, c0 + CH) - seg
                            o0 = seg - c0
                            nc.scalar.dma_start(
                                out=outT128_sb[ct_i][p0 : p0 + ln, :],
                                in_=outT_sb[o0 : o0 + ln, g, :],
                            )
                            seg += ln

                # ---- P: y = out @ w_proj.T + b_proj, stored [n, c] ----
                for t in range(NT):
                    r = _rows(t)
                    y_sb = y_p.tile([P, C], F32, tag="y")
                    for cc in range(C // COCH):
                        ps = ps_big.tile([P, COCH], F32, tag="big")
                        if proj128:
                            for ck in range(CT):
                                nc.tensor.matmul(
                                    ps[:r, :],
                                    outT128_sb[ck][:, t * P : t * P + r],
                                    wproj_sb[ck][
                                        :, cc * COCH : (cc + 1) * COCH
                                    ],
                                    start=(ck == 0),
                                    stop=(ck == CT - 1),
                                )
                        else:
                            for g in range(G):
                                nc.tensor.matmul(
                                    ps[:r, :],
                                    outT_sb[:, g, t * P : t * P + r],
                                    wproj_sb[g][
                                        :, cc * COCH : (cc + 1) * COCH
                                    ],
                                    start=(g == 0),
                                    stop=(g == G - 1),
                                )
                        if biases:
                            nc.vector.tensor_add(
                                out=y_sb[:r, cc * COCH : (cc + 1) * COCH],
                                in0=ps[:r, :],
                                in1=bias_pj[:r, cc * COCH : (cc + 1) * COCH],
                            )
                        else:
                            nc.vector.tensor_copy(
                                out=y_sb[:r, cc * COCH : (cc + 1) * COCH],
                                in_=ps[:r, :],
                            )
                    nc.sync.dma_start(
                        out=y[b, t * P : t * P + r, :], in_=y_sb[:r, :]
                    )

    nc.compile()
    return nc


_CACHE = {}
LAST_RESULTS = None


def _get_nc(mm, attn, biases=True):
    key = (mm, attn, biases)
    if key not in _CACHE:
        _CACHE[key] = build_nc(mm=mm, attn=attn, biases=biases)
    return _CACHE[key]


def _make_in_maps(inputs, mm):
    x = np.ascontiguousarray(np.asarray(inputs["x"], dtype=np.float32))
    w_qkv = np.asarray(inputs["w_qkv"], dtype=np.float32)
    b_qkv = np.ascontiguousarray(np.asarray(inputs["b_qkv"], dtype=np.float32))
    w_proj = np.asarray(inputs["w_proj"], dtype=np.float32)
    b_proj = np.ascontiguousarray(np.asarray(inputs["b_proj"], dtype=np.float32))

    wdt = np.float32
    if mm == "bf16":
        import ml_dtypes

        wdt = ml_dtypes.bfloat16
    wqkvT = np.ascontiguousarray(w_qkv.T).astype(wdt)
    wprojT = np.ascontiguousarray(w_proj.T).astype(wdt)

    return [
        {
            "x": np.ascontiguousarray(x[c * BP : (c + 1) * BP]),
            "wqkvT": wqkvT,
            "bqkv": b_qkv,
            "wprojT": wprojT,
            "bproj": b_proj,
        }
        for c in range(NCORES)
    ]


def kernel(**inputs):
    global LAST_RESULTS
    import os

    mm = os.environ.get("KERNEL_MM_DTYPE", "bf16")
    attn = os.environ.get("KERNEL_ATTN_DTYPE", "bf16")
    biases = bool(
        np.any(np.asarray(inputs["b_qkv"])) or np.any(np.asarray(inputs["b_proj"]))
    )

    nc = _get_nc(mm, attn, biases)
    in_maps = _make_in_maps(inputs, mm)
    res = run_bass_kernel_spmd(nc, in_maps, core_ids=list(range(NCORES)))
    LAST_RESULTS = res
    out = np.concatenate([r["y"] for r in res.results], axis=0)
    return out, np.asarray(inputs["size"])
